# revision 33
# baseline (speedup 1.0000x reference)
"""Differentiable episodic memory retrieval kernel for Trainium2 (8 NeuronCores).

Shards mamba_states over batch (1 batch element per core); memory matrix and
projection weights are replicated. All device tensors use a feature-major
("transposed") layout [d, tokens] so every matmul contracts over the SBUF
partition dimension.

Math (per core, X = states^T [d, tok]):
  Q^T = Wq^T X + bq                  (f32r matmuls)
  c_t = 1/||Q_t||                    (Square + ones-matmul partition reduction)
  S^T[n,t] = K^T(d,n) . Q^T(d,t)     (bf16; K unnormalized)
  E = exp(S * c_t * kscale_n)        (kscale_n = 1/(sqrt(d)*||K_n||), ACT scale)
  w = E / sum_n E                    (ones-matmul sums, K=1 matmul broadcast)
  R^T = V^T w  (V includes bv; softmax weights sum to 1 so bias passes through)
  G = Wg1^T X + W2V^T w + bg         (W2V^T = V Wg2 precomputed in preamble)
  O = R + sigmoid(G) * (X - R)
"""

import numpy as np

import concourse.bass as bass
import concourse.mybir as mybir
import concourse.tile as tile
from concourse import bacc
from concourse.bass_utils import run_bass_kernel_spmd

B, T, D = 8, 4096, 1024
NS = 512          # memory slots
TB = 512          # tokens per block
NBLK = T // TB    # 8
NDT = D // 128    # 8 tiles along d
NST = NS // 128   # 4 tiles along slots
P = 128
H = D // 2

F32 = mybir.dt.float32
F32R = mybir.dt.float32r
BF16 = mybir.dt.bfloat16

_CACHE = {}
LAST_RESULTS = None


def _f32(ap):
    return ap.bitcast(F32)


def _build():
    from contextlib import ExitStack

    nc = bacc.Bacc("TRN2", target_bir_lowering=False, debug=False)

    # f32r dram tensors: fed straight into f32r matmuls (same bits as f32)
    xt = nc.dram_tensor("xt", [D, T], F32R, kind="ExternalInput").ap()
    memt = nc.dram_tensor("memt", [D, NS], BF16, kind="ExternalInput").ap()
    wq = nc.dram_tensor("wq", [D, D], F32R, kind="ExternalInput").ap()
    wk = nc.dram_tensor("wk", [D, D], BF16, kind="ExternalInput").ap()
    wv = nc.dram_tensor("wv", [D, D], BF16, kind="ExternalInput").ap()
    wg = nc.dram_tensor("wg", [D, D], F32R, kind="ExternalInput").ap()
    wg2b = nc.dram_tensor("wg2b", [D, D], BF16, kind="ExternalInput").ap()
    bq = nc.dram_tensor("bq", [D], F32, kind="ExternalInput").ap()
    bk = nc.dram_tensor("bk", [D], F32, kind="ExternalInput").ap()
    bv = nc.dram_tensor("bv", [D], F32, kind="ExternalInput").ap()
    bg = nc.dram_tensor("bg", [D], F32, kind="ExternalInput").ap()
    ot = nc.dram_tensor("ot", [D, T], F32, kind="ExternalOutput").ap()

    with tile.TileContext(nc) as tc, ExitStack() as ctx:
        _body(nc, tc, ctx, xt, memt, wq, wk, wv, wg, wg2b, bq, bk, bv, bg, ot)

    nc.compile()
    return nc


def _body(nc, tc, ctx, xt, memt, wq, wk, wv, wg, wg2b, bq, bk, bv, bg, ot):
    Act = mybir.ActivationFunctionType

    singles = ctx.enter_context(tc.tile_pool(name="singles", bufs=1))
    wpool = ctx.enter_context(tc.tile_pool(name="weights", bufs=1))
    xpool = ctx.enter_context(tc.tile_pool(name="xt", bufs=12))

    # --- preamble-critical loads first: K projection gates the pipeline -----
    pre_cm = tc.tile_pool(name="pre", bufs=1)
    pre = pre_cm.__enter__()
    mem_sb = [pre.tile([P, NS], BF16, tag=f"mem{i}", name="mem_sb")
              for i in range(NDT)]
    for k in range(NDT):
        nc.sync.dma_start(out=mem_sb[k], in_=memt[k * P:(k + 1) * P, :])
    wk_sb = [pre.tile([P, D], BF16, tag=f"wk{i}", name="wk_sb")
             for i in range(NDT)]
    for k in range(NDT):
        nc.sync.dma_start(out=wk_sb[k], in_=wk[k * P:(k + 1) * P, :])

    # block-0 activations early so Q can fill preamble gaps
    xts0 = [xpool.tile([P, TB], F32R, tag="xt", name="xts") for _ in range(NDT)]
    for k in range(NDT):
        nc.sync.dma_start(out=xts0[k], in_=xt[k * P:(k + 1) * P, 0:TB])

    # --- constants -----------------------------------------------------------
    ones_col = singles.tile([P, 1], BF16)          # lhsT for partition sums
    nc.vector.memset(ones_col, 1.0)
    ones_row = singles.tile([1, P], F32)           # lhsT for partition bcast
    nc.vector.memset(ones_row, 1.0)
    ones_row_bf = singles.tile([1, P], BF16)
    nc.vector.memset(ones_row_bf, 1.0)

    # per-partition bias tiles: [p, t] = b[t*128 + p]
    bq_sb = singles.tile([P, NDT], F32)
    nc.sync.dma_start(out=bq_sb, in_=bq.rearrange("(t p) -> p t", p=P))
    bk_sb = singles.tile([P, NDT], F32)
    nc.sync.dma_start(out=bk_sb, in_=bk.rearrange("(t p) -> p t", p=P))
    bg_sb = singles.tile([P, NDT], F32)
    nc.sync.dma_start(out=bg_sb, in_=bg.rearrange("(t p) -> p t", p=P))
    bgn_sb = singles.tile([P, NDT], F32)
    nc.scalar.activation(out=bgn_sb, in_=bg_sb, func=Act.Copy, scale=-1.0)
    # bv broadcast across partitions: [128, D]
    bv_bc = singles.tile([P, D], F32)
    nc.sync.dma_start(
        out=bv_bc,
        in_=bass.AP(tensor=bv.tensor, offset=bv.offset, ap=[[0, P], [1, D]]),
    )

    # --- resident weights ----------------------------------------------------
    wq_sb = [wpool.tile([P, D], F32R, tag=f"wq{i}", name="wq_sb") for i in range(NDT)]
    for k in range(NDT):
        nc.sync.dma_start(out=wq_sb[k], in_=wq[k * P:(k + 1) * P, :])
    # wv/wg2 queue before wg: they reuse wk slots and are needed sooner
    wv_sb = [pre.tile([P, D], BF16, tag=f"wk{i}", name="wv_sb")
             for i in range(NDT)]
    for k in range(NDT):
        nc.sync.dma_start(out=wv_sb[k], in_=wv[k * P:(k + 1) * P, :])
    wg2_bf = [pre.tile([P, D], BF16, tag=f"wk{i}", name="wg2_bf")
              for i in range(NDT)]
    for k in range(NDT):
        nc.sync.dma_start(out=wg2_bf[k], in_=wg2b[k * P:(k + 1) * P, :])

    wg_sb = [wpool.tile([P, D], F32R, tag=f"wg{i}", name="wg_sb") for i in range(NDT)]
    for k in range(NDT):
        nc.sync.dma_start(out=wg_sb[k], in_=wg[k * P:(k + 1) * P, :])

    # static attention operands produced by the preamble
    ksb = [wpool.tile([P, NS], BF16, tag=f"ksb{i}", name="ksb") for i in range(NDT)]
    vsb = [wpool.tile([P, D], BF16, tag=f"vsb{i}", name="vsb") for i in range(NST)]
    w2vt = [wpool.tile([P, D], BF16, tag=f"w2vt{i}", name="w2vt") for i in range(NST)]
    kscale = [wpool.tile([P, 1], F32, tag=f"ksc{i}", name="kscale") for i in range(NST)]

    # =========================================================================
    # Preamble: K / V projections of the memory matrix, W2V^T = V @ Wg2
    # =========================================================================
    with tc.tile_pool(name="pre_ps", bufs=2, space="PSUM") as pre_ps, \
         tc.tile_pool(name="pre_tmp", bufs=2) as pre_tmp:
        # K^T feature-major [d, slots], bias added, cast to bf16
        for m in range(NDT):
            ps = pre_ps.tile([P, NS], F32, tag="pps", name="ps")
            for k in range(NDT):
                nc.tensor.matmul(
                    ps, wk_sb[k][:, m * P:(m + 1) * P], mem_sb[k],
                    start=(k == 0), stop=(k == NDT - 1),
                )
            nc.scalar.activation(
                out=ksb[m], in_=ps, func=Act.Identity, bias=bk_sb[:, m:m + 1],
            )

        # per-slot 1/(sqrt(d)*||K_n||) from feature-major K^T:
        # Square(ksb) -> ones-matmul over d -> [1, slots] -> PE transpose
        kss_ps = pre_ps.tile([1, NS], F32, tag="kssp", name="kss_ps")
        for m in range(NDT):
            ksq = pre_tmp.tile([P, NS], BF16, tag="ksq")
            nc.scalar.activation(out=ksq, in_=ksb[m], func=Act.Square)
            nc.tensor.matmul(kss_ps, ones_col, ksq,
                             start=(m == 0), stop=(m == NDT - 1))
        # ln/exp rsqrt on the [1, slots] row, then transpose to [128, NST]
        kroot = pre_tmp.tile([1, NS], F32, tag="kroot")
        nc.scalar.activation(out=kroot, in_=kss_ps, func=Act.Ln, scale=float(D))
        kscale_row = pre_tmp.tile([1, NS], F32, tag="kscrow")
        nc.scalar.activation(out=kscale_row, in_=kroot, func=Act.Exp, scale=-0.5)
        ident1 = pre_tmp.tile([1, 1], F32, tag="id1")
        nc.vector.memset(ident1, 1.0)
        for s in range(NST):
            kt_ps = pre_ps.tile([P, 1], F32, tag="ktp", name="kt_ps")
            nc.tensor.transpose(
                kt_ps, kscale_row[0:1, s * P:(s + 1) * P], ident1,
            )
            nc.vector.tensor_copy(out=kscale[s], in_=kt_ps)

        # V slot-major [slots, d], bias added directly (softmax weights sum to
        # one, so R = w @ (V0 + bv) = w @ V0 + bv matches the reference)
        for s in range(NST):
            vtmp = pre_tmp.tile([P, D], F32, tag="vtmp")
            for h in range(2):
                ps = pre_ps.tile([P, H], F32, tag="pps", name="ps")
                for k in range(NDT):
                    nc.tensor.matmul(
                        ps,
                        mem_sb[k][:, s * P:(s + 1) * P],
                        wv_sb[k][:, h * H:(h + 1) * H],
                        start=(k == 0), stop=(k == NDT - 1),
                    )
                nc.vector.tensor_add(
                    out=vtmp[:, h * H:(h + 1) * H], in0=ps,
                    in1=bv_bc[:, h * H:(h + 1) * H],
                )
            nc.vector.tensor_copy(out=vsb[s], in_=vtmp)

        # V^T feature-major (bf16, transient) by PE-transposing V slot-major
        identp = pre_tmp.tile([P, P], BF16, tag="idp")
        from concourse.masks import make_identity
        make_identity(nc, identp)
        vt_bf = [pre_tmp.tile([P, NS], BF16, tag=f"vt{i}", bufs=1, name="vt_bf")
                 for i in range(NDT)]
        for m in range(NDT):
            for s in range(NST):
                tp = pre_ps.tile([P, P], BF16, tag="ktp", name="tp")
                nc.tensor.transpose(
                    tp, vsb[s][:, m * P:(m + 1) * P], identp,
                )
                nc.vector.tensor_copy(
                    out=vt_bf[m][:, s * P:(s + 1) * P], in_=tp,
                )

        # W2V^T slot-major [slots, dout] = V @ Wg2   (bf16)
        for s in range(NST):
            for h in range(2):
                ps = pre_ps.tile([P, H], F32, tag="pps", name="ps")
                for k in range(NDT):
                    nc.tensor.matmul(
                        ps, vt_bf[k][:, s * P:(s + 1) * P],
                        wg2_bf[k][:, h * H:(h + 1) * H],
                        start=(k == 0), stop=(k == NDT - 1),
                    )
                nc.scalar.activation(
                    out=w2vt[s][:, h * H:(h + 1) * H], in_=ps, func=Act.Copy,
                )

    pre_cm.__exit__(None, None, None)

    # =========================================================================
    # Main loop over token blocks
    # =========================================================================
    qpool = ctx.enter_context(tc.tile_pool(name="q", bufs=10))
    rpool = ctx.enter_context(tc.tile_pool(name="r", bufs=9))
    qqpool = ctx.enter_context(tc.tile_pool(name="qq", bufs=3))
    epool = ctx.enter_context(tc.tile_pool(name="e", bufs=5))
    gpool = ctx.enter_context(tc.tile_pool(name="g", bufs=4))
    tpool = ctx.enter_context(tc.tile_pool(name="tmp", bufs=4))
    opool = ctx.enter_context(tc.tile_pool(name="o", bufs=4))
    bpool = ctx.enter_context(tc.tile_pool(name="bcast", bufs=1))
    spool = ctx.enter_context(tc.tile_pool(name="small", bufs=2))

    ps_acc = ctx.enter_context(tc.tile_pool(name="ps_acc", bufs=2, space="PSUM"))
    ps_g = ctx.enter_context(tc.tile_pool(name="ps_g", bufs=3, space="PSUM"))
    ps_s = ctx.enter_context(tc.tile_pool(name="ps_s", bufs=2, space="PSUM"))
    ps_sm = ctx.enter_context(tc.tile_pool(name="ps_sm", bufs=1, space="PSUM"))

    for blk in range(NBLK):
        col = slice(blk * TB, (blk + 1) * TB)

        if blk == 0:
            xts = xts0
        else:
            xts = [xpool.tile([P, TB], F32R, tag="xt", name="xts")
                   for _ in range(NDT)]
            for k in range(NDT):
                nc.sync.dma_start(out=xts[k], in_=xt[k * P:(k + 1) * P, col])

        # ---- Q projection + sum of squares ---------------------------------
        qss_ps = ps_sm.tile([1, TB], F32, tag="sm", name="qss_ps")
        qsb = []
        for m in range(NDT):
            ps = ps_acc.tile([P, TB], F32, tag="acc", name="ps")
            for k in range(NDT):
                nc.tensor.matmul(
                    ps, wq_sb[k][:, m * P:(m + 1) * P], xts[k],
                    start=(k == 0), stop=(k == NDT - 1),
                )
            q_m = qpool.tile([P, TB], BF16, tag="qsb", name="q_m")
            nc.scalar.activation(
                out=q_m, in_=ps, func=Act.Identity, bias=bq_sb[:, m:m + 1],
            )
            qq_m = qqpool.tile([P, TB], BF16, tag="qsq", name="qq_m")
            nc.vector.tensor_mul(out=qq_m, in0=q_m, in1=q_m)
            nc.tensor.matmul(
                qss_ps, ones_col, qq_m,
                start=(m == 0), stop=(m == NDT - 1),
            )
            qsb.append(q_m)

        # ---- per-token scale c = 1/||Q_t|| = exp(-0.5 ln(qss)) -------------
        # (ln+exp live in one ACT table set; avoids sqrt-set swaps)
        ln_q = spool.tile([1, TB], F32, tag="lnq")
        nc.scalar.activation(out=ln_q, in_=qss_ps, func=Act.Ln)
        c_sb = spool.tile([1, TB], BF16, tag="c")
        nc.scalar.activation(out=c_sb, in_=ln_q, func=Act.Exp, scale=-0.5)
        cb_ps = ps_sm.tile([P, TB], F32, tag="sm", name="cb_ps")
        nc.tensor.matmul(cb_ps, ones_row_bf, c_sb, start=True, stop=True)
        cb_sb = bpool.tile([P, TB], F32, tag="cb_sb")
        nc.scalar.activation(out=cb_sb, in_=cb_ps, func=Act.Copy)

        # ---- scores^T [slots, tok] -> exp (s-outer: 2 psum banks) ----------
        ss_ps = ps_sm.tile([1, TB], F32, tag="sm", name="ss_ps")
        esb = []
        for s in range(NST):
            s_ps = ps_s.tile([P, TB], F32, tag="pss", name="s_ps")
            for m in range(NDT):
                nc.tensor.matmul(
                    s_ps, ksb[m][:, s * P:(s + 1) * P], qsb[m],
                    start=(m == 0), stop=(m == NDT - 1),
                )
            e_in = tpool.tile([P, TB], F32, tag="tt", name="e_in")
            nc.vector.tensor_mul(out=e_in, in0=s_ps, in1=cb_sb)
            e_s = epool.tile([P, TB], BF16, tag="esb", name="e_s")
            nc.scalar.activation(out=e_s, in_=e_in, func=Act.Exp, scale=kscale[s])
            nc.tensor.matmul(
                ss_ps, ones_col, e_s, start=(s == 0), stop=(s == NST - 1),
            )
            esb.append(e_s)

        # ---- gate X-half: no softmax dependency, fills the softmax gap -----
        g_ps = []
        for m in range(NDT):
            ps = ps_g.tile([P, TB], F32, tag="psg", name="ps")
            for k in range(NDT):
                nc.tensor.matmul(
                    ps, wg_sb[k][:, m * P:(m + 1) * P], xts[k],
                    start=(k == 0), stop=False,
                )
            g_ps.append(ps)

        # ---- normalized weights w = E / sum --------------------------------
        rs_sb = spool.tile([1, TB], F32, tag="rs")
        rs_scr = spool.tile([1, TB], F32, tag="rs_scr")
        nc.vector.reciprocal_approx_accurate(out=rs_sb, in_=ss_ps, scratch=rs_scr)
        rb_ps = ps_sm.tile([P, TB], F32, tag="sm", name="rb_ps")
        nc.tensor.matmul(rb_ps, ones_row, rs_sb, start=True, stop=True)
        wsb = []
        for s in range(NST):
            w_s = epool.tile([P, TB], BF16, tag="wsb", name="w_s")
            nc.vector.tensor_mul(out=w_s, in0=rb_ps, in1=esb[s])
            wsb.append(w_s)

        # ---- R^T = V^T w  [d, tok]; DVE-copy to SBUF frees the acc slot ----
        # quickly so the next block's Q matmuls can claim it early.
        rsb = []
        for m in range(NDT):
            ps = ps_acc.tile([P, TB], F32, tag="acc", name="ps")
            for s in range(NST):
                nc.tensor.matmul(
                    ps, vsb[s][:, m * P:(m + 1) * P], wsb[s],
                    start=(s == 0), stop=(s == NST - 1),
                )
            r_m = rpool.tile([P, TB], F32, tag="rsb", name="r_m")
            nc.vector.tensor_copy(out=r_m, in_=ps)
            rsb.append(r_m)

        # ---- gate completion (W2V^T w) + sigmoid + blend --------------------
        for m in range(NDT):
            ps = g_ps[m]
            for s in range(NST):
                nc.tensor.matmul(
                    ps, w2vt[s][:, m * P:(m + 1) * P], wsb[s],
                    start=False, stop=(s == NST - 1),
                )
            eg_m = gpool.tile([P, TB], F32, tag="egs", name="eg_m")
            nc.scalar.activation(
                out=eg_m, in_=ps, func=Act.Exp, scale=-1.0,
                bias=bgn_sb[:, m:m + 1],
            )
            nc.vector.tensor_scalar_add(out=eg_m, in0=eg_m, scalar1=1.0)
            g_m = gpool.tile([P, TB], F32, tag="gsb", name="g_m")
            nc.vector.reciprocal_approx_fast(out=g_m, in_=eg_m)
            # O = R + g * (X - R)
            d_m = tpool.tile([P, TB], F32, tag="tt", name="d_m")
            nc.gpsimd.tensor_sub(out=d_m, in0=_f32(xts[m]), in1=rsb[m])
            nc.vector.tensor_mul(out=d_m, in0=d_m, in1=g_m)
            o_m = opool.tile([P, TB], F32, tag="osb", name="o_m")
            nc.vector.tensor_add(out=o_m, in0=d_m, in1=rsb[m])
            nc.sync.dma_start(out=ot[m * P:(m + 1) * P, col], in_=o_m)


def kernel(mamba_states, memory, Wq, bq, Wk, bk, Wv, bv, Wg, bg):
    global LAST_RESULTS
    if "nc" not in _CACHE:
        _CACHE["nc"] = _build()
    nc = _CACHE["nc"]

    import ml_dtypes

    f = np.ascontiguousarray
    b16 = ml_dtypes.bfloat16
    wg_f = np.asarray(Wg, np.float32)
    shared = {
        "memt": f(np.asarray(memory, np.float32).T.astype(b16)),
        "wq": f(np.asarray(Wq, np.float32)),
        "wk": f(np.asarray(Wk, np.float32).astype(b16)),
        "wv": f(np.asarray(Wv, np.float32).astype(b16)),
        "wg": f(wg_f[:D]),
        "wg2b": f(wg_f[D:].astype(b16)),
        "bq": f(np.asarray(bq, np.float32)),
        "bk": f(np.asarray(bk, np.float32)),
        "bv": f(np.asarray(bv, np.float32)),
        "bg": f(np.asarray(bg, np.float32)),
    }
    states = np.asarray(mamba_states, np.float32)
    in_maps = [dict(shared, xt=f(states[i].T)) for i in range(B)]

    res = run_bass_kernel_spmd(
        nc, in_maps, list(range(B)), trace=_CACHE.get("trace", False)
    )
    LAST_RESULTS = res
    out = np.stack([res.results[i]["ot"].T for i in range(B)])
    return np.ascontiguousarray(out)


# revision 39
# speedup vs baseline: 1.0852x; 1.0852x over previous
"""Differentiable episodic memory retrieval kernel for Trainium2 (8 NeuronCores).

Shards mamba_states over batch (1 batch element per core); memory matrix and
projection weights are replicated. All device tensors use a feature-major
("transposed") layout [d, tokens] so every matmul contracts over the SBUF
partition dimension.

Math (per core, X = states^T [d, tok]):
  Q^T = Wq^T X + bq                  (f32r matmuls)
  c_t = 1/||Q_t||                    (Square + ones-matmul partition reduction)
  S^T[n,t] = K^T(d,n) . Q^T(d,t)     (bf16; K unnormalized)
  E = exp(S * c_t * kscale_n)        (kscale_n = 1/(sqrt(d)*||K_n||), ACT scale)
  w = E / sum_n E                    (ones-matmul sums, K=1 matmul broadcast)
  R^T = V^T w  (V includes bv; softmax weights sum to 1 so bias passes through)
  G = Wg1^T X + W2V^T w + bg         (W2V^T = V Wg2 precomputed in preamble)
  O = R + sigmoid(G) * (X - R)
"""

import numpy as np

import concourse.bass as bass
import concourse.mybir as mybir
import concourse.tile as tile
from concourse import bacc
from concourse.bass_utils import run_bass_kernel_spmd

B, T, D = 8, 4096, 1024
NS = 512          # memory slots
TB = 512          # tokens per block
NBLK = T // TB    # 8
NDT = D // 128    # 8 tiles along d
NST = NS // 128   # 4 tiles along slots
P = 128
H = D // 2

F32 = mybir.dt.float32
F32R = mybir.dt.float32r
BF16 = mybir.dt.bfloat16

_CACHE = {}
LAST_RESULTS = None


def _f32(ap):
    return ap.bitcast(F32)


def _build():
    from contextlib import ExitStack

    nc = bacc.Bacc("TRN2", target_bir_lowering=False, debug=False)

    # f32r dram tensors: fed straight into f32r matmuls (same bits as f32)
    xt = nc.dram_tensor("xt", [D, T], F32R, kind="ExternalInput").ap()
    memt = nc.dram_tensor("memt", [D, NS], BF16, kind="ExternalInput").ap()
    wq = nc.dram_tensor("wq", [D, D], F32R, kind="ExternalInput").ap()
    wk = nc.dram_tensor("wk", [D, D], BF16, kind="ExternalInput").ap()
    wv = nc.dram_tensor("wv", [D, D], BF16, kind="ExternalInput").ap()
    wg = nc.dram_tensor("wg", [D, D], F32R, kind="ExternalInput").ap()
    wg2b = nc.dram_tensor("wg2b", [D, D], BF16, kind="ExternalInput").ap()
    bq = nc.dram_tensor("bq", [D], F32, kind="ExternalInput").ap()
    bk = nc.dram_tensor("bk", [D], F32, kind="ExternalInput").ap()
    bv = nc.dram_tensor("bv", [D], F32, kind="ExternalInput").ap()
    bg = nc.dram_tensor("bg", [D], F32, kind="ExternalInput").ap()
    ot = nc.dram_tensor("ot", [D, T], F32, kind="ExternalOutput").ap()

    with tile.TileContext(nc) as tc, ExitStack() as ctx:
        _body(nc, tc, ctx, xt, memt, wq, wk, wv, wg, wg2b, bq, bk, bv, bg, ot)

    nc.compile()
    return nc


def _body(nc, tc, ctx, xt, memt, wq, wk, wv, wg, wg2b, bq, bk, bv, bg, ot):
    Act = mybir.ActivationFunctionType

    singles = ctx.enter_context(tc.tile_pool(name="singles", bufs=1))
    wpool = ctx.enter_context(tc.tile_pool(name="weights", bufs=1))
    xpool = ctx.enter_context(tc.tile_pool(name="xt", bufs=12))

    # --- preamble-critical loads first: K projection gates the pipeline -----
    pre_cm = tc.tile_pool(name="pre", bufs=1)
    pre = pre_cm.__enter__()
    mem_sb = [pre.tile([P, NS], BF16, tag=f"mem{i}", name="mem_sb")
              for i in range(NDT)]
    for k in range(NDT):
        nc.sync.dma_start(out=mem_sb[k], in_=memt[k * P:(k + 1) * P, :])
    wk_sb = [pre.tile([P, D], BF16, tag=f"wk{i}", name="wk_sb")
             for i in range(NDT)]
    for k in range(NDT):
        nc.sync.dma_start(out=wk_sb[k], in_=wk[k * P:(k + 1) * P, :])

    # block-0 activations early so Q can fill preamble gaps
    xts0 = [xpool.tile([P, TB], F32R, tag="xt", name="xts") for _ in range(NDT)]
    for k in range(NDT):
        nc.sync.dma_start(out=xts0[k], in_=xt[k * P:(k + 1) * P, 0:TB])

    # --- constants -----------------------------------------------------------
    ones_col = singles.tile([P, 1], BF16)          # lhsT for partition sums
    nc.vector.memset(ones_col, 1.0)
    ones_row = singles.tile([1, P], F32)           # lhsT for partition bcast
    nc.vector.memset(ones_row, 1.0)
    ones_row_bf = singles.tile([1, P], BF16)
    nc.vector.memset(ones_row_bf, 1.0)

    # per-partition bias tiles: [p, t] = b[t*128 + p]
    bq_sb = singles.tile([P, NDT], F32)
    nc.sync.dma_start(out=bq_sb, in_=bq.rearrange("(t p) -> p t", p=P))
    bk_sb = singles.tile([P, NDT], F32)
    nc.sync.dma_start(out=bk_sb, in_=bk.rearrange("(t p) -> p t", p=P))
    bg_sb = singles.tile([P, NDT], F32)
    nc.sync.dma_start(out=bg_sb, in_=bg.rearrange("(t p) -> p t", p=P))
    bgn_sb = singles.tile([P, NDT], F32)
    nc.scalar.activation(out=bgn_sb, in_=bg_sb, func=Act.Copy, scale=-1.0)
    # bv broadcast across partitions: [128, D]
    bv_bc = singles.tile([P, D], F32)
    nc.sync.dma_start(
        out=bv_bc,
        in_=bass.AP(tensor=bv.tensor, offset=bv.offset, ap=[[0, P], [1, D]]),
    )

    # --- resident weights ----------------------------------------------------
    wq_sb = [wpool.tile([P, D], F32R, tag=f"wq{i}", name="wq_sb") for i in range(NDT)]
    for k in range(NDT):
        nc.sync.dma_start(out=wq_sb[k], in_=wq[k * P:(k + 1) * P, :])
    # wv/wg2 queue before wg: they reuse wk slots and are needed sooner
    wv_sb = [pre.tile([P, D], BF16, tag=f"wk{i}", name="wv_sb")
             for i in range(NDT)]
    for k in range(NDT):
        nc.sync.dma_start(out=wv_sb[k], in_=wv[k * P:(k + 1) * P, :])
    wg2_bf = [pre.tile([P, D], BF16, tag=f"wk{i}", name="wg2_bf")
              for i in range(NDT)]
    for k in range(NDT):
        nc.sync.dma_start(out=wg2_bf[k], in_=wg2b[k * P:(k + 1) * P, :])

    wg_sb = [wpool.tile([P, D], F32R, tag=f"wg{i}", name="wg_sb") for i in range(NDT)]
    for k in range(NDT):
        nc.sync.dma_start(out=wg_sb[k], in_=wg[k * P:(k + 1) * P, :])

    # static attention operands produced by the preamble
    ksb = [wpool.tile([P, NS], BF16, tag=f"ksb{i}", name="ksb") for i in range(NDT)]
    vsb = [wpool.tile([P, D], BF16, tag=f"vsb{i}", name="vsb") for i in range(NST)]
    w2vt = [wpool.tile([P, D], BF16, tag=f"w2vt{i}", name="w2vt") for i in range(NST)]
    kscale = [wpool.tile([P, 1], F32, tag=f"ksc{i}", name="kscale") for i in range(NST)]

    # =========================================================================
    # Preamble: K / V projections of the memory matrix, W2V^T = V @ Wg2
    # =========================================================================
    with tc.tile_pool(name="pre_ps", bufs=2, space="PSUM") as pre_ps, \
         tc.tile_pool(name="pre_tmp", bufs=2) as pre_tmp:
        # K^T feature-major [d, slots], bias added, cast to bf16
        for m in range(NDT):
            ps = pre_ps.tile([P, NS], F32, tag="pps", name="ps")
            for k in range(NDT):
                nc.tensor.matmul(
                    ps, wk_sb[k][:, m * P:(m + 1) * P], mem_sb[k],
                    start=(k == 0), stop=(k == NDT - 1),
                )
            nc.scalar.activation(
                out=ksb[m], in_=ps, func=Act.Identity, bias=bk_sb[:, m:m + 1],
            )

        # per-slot 1/(sqrt(d)*||K_n||) from feature-major K^T:
        # Square(ksb) -> ones-matmul over d -> [1, slots] -> PE transpose
        kss_ps = pre_ps.tile([1, NS], F32, tag="kssp", name="kss_ps")
        for m in range(NDT):
            ksq = pre_tmp.tile([P, NS], BF16, tag="ksq")
            nc.scalar.activation(out=ksq, in_=ksb[m], func=Act.Square)
            nc.tensor.matmul(kss_ps, ones_col, ksq,
                             start=(m == 0), stop=(m == NDT - 1))
        # ln/exp rsqrt on the [1, slots] row, then transpose to [128, NST]
        kroot = pre_tmp.tile([1, NS], F32, tag="kroot")
        nc.scalar.activation(out=kroot, in_=kss_ps, func=Act.Ln, scale=float(D))
        kscale_row = pre_tmp.tile([1, NS], F32, tag="kscrow")
        nc.scalar.activation(out=kscale_row, in_=kroot, func=Act.Exp, scale=-0.5)
        ident1 = pre_tmp.tile([1, 1], F32, tag="id1")
        nc.vector.memset(ident1, 1.0)
        for s in range(NST):
            kt_ps = pre_ps.tile([P, 1], F32, tag="ktp", name="kt_ps")
            nc.tensor.transpose(
                kt_ps, kscale_row[0:1, s * P:(s + 1) * P], ident1,
            )
            nc.vector.tensor_copy(out=kscale[s], in_=kt_ps)

        # V slot-major [slots, d], bias added directly (softmax weights sum to
        # one, so R = w @ (V0 + bv) = w @ V0 + bv matches the reference)
        for s in range(NST):
            vtmp = pre_tmp.tile([P, D], F32, tag="vtmp")
            for h in range(2):
                ps = pre_ps.tile([P, H], F32, tag="pps", name="ps")
                for k in range(NDT):
                    nc.tensor.matmul(
                        ps,
                        mem_sb[k][:, s * P:(s + 1) * P],
                        wv_sb[k][:, h * H:(h + 1) * H],
                        start=(k == 0), stop=(k == NDT - 1),
                    )
                nc.vector.tensor_add(
                    out=vtmp[:, h * H:(h + 1) * H], in0=ps,
                    in1=bv_bc[:, h * H:(h + 1) * H],
                )
            nc.vector.tensor_copy(out=vsb[s], in_=vtmp)

        # V^T feature-major (bf16, transient) by PE-transposing V slot-major
        identp = pre_tmp.tile([P, P], BF16, tag="idp")
        from concourse.masks import make_identity
        make_identity(nc, identp)
        vt_bf = [pre_tmp.tile([P, NS], BF16, tag=f"vt{i}", bufs=1, name="vt_bf")
                 for i in range(NDT)]
        for m in range(NDT):
            for s in range(NST):
                tp = pre_ps.tile([P, P], BF16, tag="ktp", name="tp")
                nc.tensor.transpose(
                    tp, vsb[s][:, m * P:(m + 1) * P], identp,
                )
                nc.vector.tensor_copy(
                    out=vt_bf[m][:, s * P:(s + 1) * P], in_=tp,
                )

        # W2V^T slot-major [slots, dout] = V @ Wg2   (bf16)
        for s in range(NST):
            for h in range(2):
                ps = pre_ps.tile([P, H], F32, tag="pps", name="ps")
                for k in range(NDT):
                    nc.tensor.matmul(
                        ps, vt_bf[k][:, s * P:(s + 1) * P],
                        wg2_bf[k][:, h * H:(h + 1) * H],
                        start=(k == 0), stop=(k == NDT - 1),
                    )
                nc.scalar.activation(
                    out=w2vt[s][:, h * H:(h + 1) * H], in_=ps, func=Act.Copy,
                )

    pre_cm.__exit__(None, None, None)

    # =========================================================================
    # Main loop over token blocks
    # =========================================================================
    qpool = ctx.enter_context(tc.tile_pool(name="q", bufs=10))
    rpool = ctx.enter_context(tc.tile_pool(name="r", bufs=9))
    qqpool = ctx.enter_context(tc.tile_pool(name="qq", bufs=3))
    epool = ctx.enter_context(tc.tile_pool(name="e", bufs=5))
    gpool = ctx.enter_context(tc.tile_pool(name="g", bufs=4))
    tpool = ctx.enter_context(tc.tile_pool(name="tmp", bufs=4))
    opool = ctx.enter_context(tc.tile_pool(name="o", bufs=4))
    bpool = ctx.enter_context(tc.tile_pool(name="bcast", bufs=1))
    spool = ctx.enter_context(tc.tile_pool(name="small", bufs=2))

    ps_acc = ctx.enter_context(tc.tile_pool(name="ps_acc", bufs=2, space="PSUM"))
    ps_g = ctx.enter_context(tc.tile_pool(name="ps_g", bufs=3, space="PSUM"))
    ps_s = ctx.enter_context(tc.tile_pool(name="ps_s", bufs=2, space="PSUM"))
    ps_sm = ctx.enter_context(tc.tile_pool(name="ps_sm", bufs=1, space="PSUM"))

    for blk in range(NBLK):
        col = slice(blk * TB, (blk + 1) * TB)

        if blk == 0:
            xts = xts0
        else:
            xts = [xpool.tile([P, TB], F32R, tag="xt", name="xts")
                   for _ in range(NDT)]
            for k in range(NDT):
                nc.sync.dma_start(out=xts[k], in_=xt[k * P:(k + 1) * P, col])

        # ---- Q projection + sum of squares ---------------------------------
        qss_ps = ps_sm.tile([1, TB], F32, tag="sm", name="qss_ps")
        qsb = []
        for m in range(NDT):
            ps = ps_acc.tile([P, TB], F32, tag="acc", name="ps")
            for k in range(NDT):
                nc.tensor.matmul(
                    ps, wq_sb[k][:, m * P:(m + 1) * P], xts[k],
                    start=(k == 0), stop=(k == NDT - 1),
                )
            q_m = qpool.tile([P, TB], BF16, tag="qsb", name="q_m")
            nc.scalar.activation(
                out=q_m, in_=ps, func=Act.Identity, bias=bq_sb[:, m:m + 1],
            )
            qq_m = qqpool.tile([P, TB], BF16, tag="qsq", name="qq_m")
            nc.vector.tensor_mul(out=qq_m, in0=q_m, in1=q_m)
            nc.tensor.matmul(
                qss_ps, ones_col, qq_m,
                start=(m == 0), stop=(m == NDT - 1),
            )
            qsb.append(q_m)

        # ---- per-token scale c = 1/||Q_t|| = exp(-0.5 ln(qss)) -------------
        # (ln+exp live in one ACT table set; avoids sqrt-set swaps)
        ln_q = spool.tile([1, TB], F32, tag="lnq")
        nc.scalar.activation(out=ln_q, in_=qss_ps, func=Act.Ln)
        c_sb = spool.tile([1, TB], BF16, tag="c")
        nc.scalar.activation(out=c_sb, in_=ln_q, func=Act.Exp, scale=-0.5)
        cb_ps = ps_sm.tile([P, TB], F32, tag="sm", name="cb_ps")
        nc.tensor.matmul(cb_ps, ones_row_bf, c_sb, start=True, stop=True)
        cb_sb = bpool.tile([P, TB], F32, tag="cb_sb")
        nc.scalar.activation(out=cb_sb, in_=cb_ps, func=Act.Copy)

        # ---- scores^T [slots, tok] -> exp (s-outer: 2 psum banks) ----------
        ss_ps = ps_sm.tile([1, TB], F32, tag="sm", name="ss_ps")
        esb = []
        for s in range(NST):
            s_ps = ps_s.tile([P, TB], F32, tag="pss", name="s_ps")
            for m in range(NDT):
                nc.tensor.matmul(
                    s_ps, ksb[m][:, s * P:(s + 1) * P], qsb[m],
                    start=(m == 0), stop=(m == NDT - 1),
                )
            e_in = tpool.tile([P, TB], F32, tag="tt", name="e_in")
            nc.vector.tensor_mul(out=e_in, in0=s_ps, in1=cb_sb)
            e_s = epool.tile([P, TB], BF16, tag="esb", name="e_s")
            nc.scalar.activation(out=e_s, in_=e_in, func=Act.Exp, scale=kscale[s])
            nc.tensor.matmul(
                ss_ps, ones_col, e_s, start=(s == 0), stop=(s == NST - 1),
            )
            esb.append(e_s)

        # ---- gate X-half: no softmax dependency, fills the softmax gap -----
        g_ps = []
        for m in range(NDT):
            ps = ps_g.tile([P, TB], F32, tag="psg", name="ps")
            for k in range(NDT):
                nc.tensor.matmul(
                    ps, wg_sb[k][:, m * P:(m + 1) * P], xts[k],
                    start=(k == 0), stop=False,
                )
            g_ps.append(ps)

        # ---- normalized weights w = E / sum --------------------------------
        rs_sb = spool.tile([1, TB], F32, tag="rs")
        rs_scr = spool.tile([1, TB], F32, tag="rs_scr")
        nc.vector.reciprocal_approx_accurate(out=rs_sb, in_=ss_ps, scratch=rs_scr)
        rb_ps = ps_sm.tile([P, TB], F32, tag="sm", name="rb_ps")
        nc.tensor.matmul(rb_ps, ones_row, rs_sb, start=True, stop=True)
        wsb = []
        for s in range(NST):
            w_s = epool.tile([P, TB], BF16, tag="wsb", name="w_s")
            nc.vector.tensor_mul(out=w_s, in0=rb_ps, in1=esb[s])
            wsb.append(w_s)

        # ---- R^T = V^T w  [d, tok]; DVE-copy to SBUF frees the acc slot ----
        # quickly so the next block's Q matmuls can claim it early.
        rsb = []
        for m in range(NDT):
            ps = ps_acc.tile([P, TB], F32, tag="acc", name="ps")
            for s in range(NST):
                nc.tensor.matmul(
                    ps, vsb[s][:, m * P:(m + 1) * P], wsb[s],
                    start=(s == 0), stop=(s == NST - 1),
                )
            r_m = rpool.tile([P, TB], F32, tag="rsb", name="r_m")
            nc.vector.tensor_copy(out=r_m, in_=ps)
            rsb.append(r_m)

        # ---- gate completion (W2V^T w) + sigmoid + blend --------------------
        for m in range(NDT):
            ps = g_ps[m]
            for s in range(NST):
                nc.tensor.matmul(
                    ps, w2vt[s][:, m * P:(m + 1) * P], wsb[s],
                    start=False, stop=(s == NST - 1),
                )
            eg_m = gpool.tile([P, TB], F32, tag="egs", name="eg_m")
            nc.scalar.activation(
                out=eg_m, in_=ps, func=Act.Exp, scale=-1.0,
                bias=bgn_sb[:, m:m + 1],
            )
            nc.vector.tensor_scalar_add(out=eg_m, in0=eg_m, scalar1=1.0)
            g_m = gpool.tile([P, TB], F32, tag="gsb", name="g_m")
            nc.vector.reciprocal_approx_fast(out=g_m, in_=eg_m)
            # O = R + g * (X - R)
            d_m = tpool.tile([P, TB], F32, tag="tt", name="d_m")
            nc.gpsimd.tensor_sub(out=d_m, in0=_f32(xts[m]), in1=rsb[m])
            nc.vector.tensor_mul(out=d_m, in0=d_m, in1=g_m)
            o_m = opool.tile([P, TB], F32, tag="osb", name="o_m")
            nc.vector.tensor_add(out=o_m, in0=d_m, in1=rsb[m])
            nc.sync.dma_start(out=ot[m * P:(m + 1) * P, col], in_=o_m)


def kernel(mamba_states, memory, Wq, bq, Wk, bk, Wv, bv, Wg, bg):
    global LAST_RESULTS
    if "nc" not in _CACHE:
        _CACHE["nc"] = _build()
    nc = _CACHE["nc"]

    import ml_dtypes

    f = np.ascontiguousarray
    b16 = ml_dtypes.bfloat16
    wg_f = np.asarray(Wg, np.float32)
    shared = {
        "memt": f(np.asarray(memory, np.float32).T.astype(b16)),
        "wq": f(np.asarray(Wq, np.float32)),
        "wk": f(np.asarray(Wk, np.float32).astype(b16)),
        "wv": f(np.asarray(Wv, np.float32).astype(b16)),
        "wg": f(wg_f[:D]),
        "wg2b": f(wg_f[D:].astype(b16)),
        "bq": f(np.asarray(bq, np.float32)),
        "bk": f(np.asarray(bk, np.float32)),
        "bv": f(np.asarray(bv, np.float32)),
        "bg": f(np.asarray(bg, np.float32)),
    }
    states = np.asarray(mamba_states, np.float32)
    in_maps = [dict(shared, xt=f(states[i].T)) for i in range(B)]

    res = run_bass_kernel_spmd(
        nc, in_maps, list(range(B)), trace=_CACHE.get("trace", False)
    )
    LAST_RESULTS = res
    out = np.stack([res.results[i]["ot"].T for i in range(B)])
    return np.ascontiguousarray(out)
